# revision 6
# baseline (speedup 1.0000x reference)
"""Trainium2 Bass kernel for nn_MultiHeadDistanceLayer.

Math: out[b,k,h] = pool3(S[h,b,:])[k] where
  S[h,b,k'] = sum_{q>=k'} v[h,b,q] * softmax(QK^T/sqrt(D))[q,k']
(the final sum over the query axis commutes with the W=3 key-axis average
pool, so the device only produces the length-L vector S per (head, batch);
pooling/normalization is a trivial host epilogue).

Sharding: 16 (head, batch) pairs; core c handles batch c//4 and heads
(2*(c%4), 2*(c%4)+1). The tiny O(L*C*D) Q/K/v projections run on the host
(0.8% of FLOPs); the device does the O(L^2) work: scores, softmax, and
causal weighted column sums.

Device pipeline (v2, ScalarE-bound design): PSUM is organized as a 7-bank
ring [128, 3584] that the score matmuls (bf16, contraction zero-padded to
K=128 for the HAM clock gate) fill in 512-col chunks, while ScalarE drains
it with large alternating [128,1536]/[128,2048] exp instructions into a
single contiguous bf16 E buffer (this amortizes the ~500-cycle per-
instruction ACT overhead that dominates a per-tile 2x[128,1024] scheme).
The softmax denominator Z is computed entirely off ScalarE: a 1-input DVE
tensor_scalar copy with accum_out sums each row at the 2x bf16 rate.
Causal column sums are M=1 matmuls (lhsT = v/Z column) accumulating in the
8th PSUM bank, lagged two tiles so the PE never stalls on the DVE w-chain.
"""

import sys

for _p in ("/opt/trn_rl_repo",):
    if _p not in sys.path:
        sys.path.insert(0, _p)

import numpy as np

B, L, C = 2, 2048, 256
H, D, W = 8, 32, 3
NCORES = 8
NT = L // 128          # 16 q-tiles per head
NCH = L // 512         # 4 key chunks per row
SCALE = float(D) ** -0.5
CH = 512
RING = 7               # PSUM ring slots (banks); bank 8 = colsum acc

TRACE = False
LAST_EXEC_NS = None
_COMPILED = None


def _window_boundaries():
    """Exp-window end boundaries over the 128-chunk global stream:
    alternating 3/4-chunk windows aligned to the 7-slot ring, final
    partial window at 128."""
    bnds = []
    b = 0
    while b < 126:
        b += 3
        bnds.append(b)
        b += 4
        bnds.append(b)
    bnds.append(128)
    return bnds


def _build():
    import concourse.bacc as bacc
    import concourse.tile as tile
    from concourse import mybir

    f32 = mybir.dt.float32
    bf16 = mybir.dt.bfloat16
    AF = mybir.ActivationFunctionType
    ALU = mybir.AluOpType

    nc = bacc.Bacc("TRN2", target_bir_lowering=False, debug=False,
                   num_devices=NCORES)

    # host-projected Q/K, transposed + bf16: rows [QT_h0, KT_h0, QT_h1, KT_h1]
    qk4 = nc.dram_tensor("qk4", [4, 32, L], bf16, kind="ExternalInput")
    vnat = nc.dram_tensor("vnat", [128, 2 * NT], f32, kind="ExternalInput")
    # 4 causal masks for the diagonal-containing 512-chunk; variant r=t%4
    # keeps column j (of the chunk) iff j <= 128*r + p.
    msk = nc.dram_tensor("msk", [128, 4, 512], bf16, kind="ExternalInput")
    # per head: acc rows 32c hold S for key chunk c
    sout = nc.dram_tensor("sout", [2, 4, 512], f32, kind="ExternalOutput")

    bnds = _window_boundaries()

    with tile.TileContext(nc) as tc:
        with (
            tc.tile_pool(name="big", bufs=1) as big,
            tc.tile_pool(name="qkp", bufs=2) as qkp,
            tc.tile_pool(name="zpool", bufs=2) as zpool,
            tc.tile_pool(name="empool", bufs=4) as empool,
            tc.tile_pool(name="small", bufs=16) as small,
            tc.tile_pool(name="ssbp", bufs=2) as ssbp,
            tc.tile_pool(name="pring", bufs=1, space="PSUM") as pring,
            tc.tile_pool(name="psacc", bufs=1, space="PSUM") as psacc,
        ):
            # --- per-head K-padded Q/K scratch, zeroed first (rows 32+
            # must be zero; K=32 matmuls do not register as PE activity
            # for the HAM clock gate, K=128 do) ---
            qkts = []
            for hh in range(2):
                qts = qkp.tile([128, L], bf16, tag=f"qts{hh}", name=f"qts{hh}")
                kts = qkp.tile([128, L], bf16, tag=f"kts{hh}", name=f"kts{hh}")
                qkts.append((qts, kts))

            # --- exp table preload (hidden under input DMA) ---
            warm = big.tile([128, 1], f32, tag="warm")
            nc.vector.memset(warm, 0.0)
            nc.scalar.activation(out=warm, in_=warm, func=AF.Exp)
            # --- PE warmup: ~4us of dense K=128 matmuls during the DMA
            # wait trips the HAM activity window so the first real tiles
            # run at 2.4GHz instead of 1.2GHz
            wrmt = big.tile([128, 512], bf16, tag="wrmt")
            nc.gpsimd.memset(wrmt.bitcast(mybir.dt.uint32), 0)
            wrmp = psacc.tile([128, 512], f32, tag="acc", name="wrmp")
            for i in range(9):
                nc.tensor.matmul(wrmp, wrmt[:, 0:128], wrmt,
                                 start=True, stop=True)

            # --- zero the K-pad rows, then DMA Q/K into rows 0-31 ---
            nc.vector.memset(qkts[0][1].bitcast(mybir.dt.uint32), 0)
            nc.gpsimd.memset(qkts[0][0].bitcast(mybir.dt.uint32), 0)
            nc.sync.dma_start(out=qkts[0][1][0:32, :], in_=qk4[1])
            nc.scalar.dma_start(out=qkts[0][0][0:32, :], in_=qk4[0])
            nc.vector.memset(qkts[1][1].bitcast(mybir.dt.uint32), 0)
            nc.gpsimd.memset(qkts[1][0].bitcast(mybir.dt.uint32), 0)
            nc.sync.dma_start(out=qkts[1][1][0:32, :], in_=qk4[3])
            nc.scalar.dma_start(out=qkts[1][0][0:32, :], in_=qk4[2])
            vnat_sb = big.tile([128, 2 * NT], f32, tag="vnat")
            nc.gpsimd.dma_start(out=vnat_sb, in_=vnat[:, :])
            msk_sb = big.tile([128, 4, 512], bf16, tag="msk")
            nc.gpsimd.dma_start(out=msk_sb, in_=msk[:, :, :])

            # --- big shared buffers ---
            ering = big.tile([128, 128 * CH], bf16, tag="ering")  # exp(scores)
            ring = pring.tile([128, RING * CH], f32, tag="ring")
            acc = psacc.tile([128, 512], f32, tag="acc", name="acc")

            st = {"ws": 0, "tl_post": 0}
            pend = []

            def issue_window(we):
                s0 = st["ws"] % RING
                n = we - st["ws"]
                nc.scalar.activation(
                    out=ering[:, CH * st["ws"]:CH * we],
                    in_=ring[:, CH * s0:CH * (s0 + n)],
                    func=AF.Exp, scale=SCALE)
                st["ws"] = we

            def post_tiles():
                # per-tile epilogue for tiles fully covered by issued windows
                while st["tl_post"] * 4 + 4 <= st["ws"]:
                    tl = st["tl_post"]
                    st["tl_post"] += 1
                    hh, t = tl // NT, tl % NT
                    et = ering[:, CH * 4 * tl:CH * 4 * (tl + 1)]
                    # Z via 1-input DVE pass at 2x bf16 rate (accum_out)
                    zscr = zpool.tile([128, L], bf16, tag="zscr")
                    z = small.tile([128, 1], f32, tag="z")
                    nc.vector.tensor_scalar(out=zscr, in0=et, scalar1=1.0,
                                            scalar2=None, op0=ALU.mult,
                                            op1=ALU.add, accum_out=z)
                    zr = small.tile([128, 1], f32, tag="zr")
                    nc.vector.reciprocal(zr, z)
                    w = small.tile([128, 1], bf16, tag="w")
                    nc.vector.tensor_scalar(out=w, in0=vnat_sb[:, tl:tl + 1],
                                            scalar1=zr, scalar2=None,
                                            op0=ALU.mult)
                    cb = t // 4
                    em = empool.tile([128, 512], bf16, tag="em")
                    nc.vector.tensor_mul(
                        em, ering[:, CH * (4 * tl + cb):CH * (4 * tl + cb + 1)],
                        msk_sb[:, t % 4, :])
                    pend.append((hh, t, w, em))

            def colsum(work):
                hh, t, w, em = work
                cb = t // 4
                for c2 in range(cb + 1):
                    g = 64 * hh + 4 * t + c2
                    rhs = em if c2 == cb else ering[:, CH * g:CH * (g + 1)]
                    nc.tensor.matmul(acc[32 * c2:32 * c2 + 1, :], w, rhs,
                                     start=(t == 4 * c2), stop=(t == NT - 1),
                                     tile_position=(0, 32 * c2),
                                     skip_group_check=True)
                if t == NT - 1:
                    # copy the 4 written acc rows out and ship; split across
                    # DVE and ACT so the end-of-kernel tail stays short
                    ssb = ssbp.tile([128, 512], f32, tag="ssb")
                    for c2 in range(NCH):
                        src = acc[32 * c2:32 * c2 + 1, :]
                        dst = ssb[32 * c2:32 * c2 + 1, :]
                        if c2 % 2 == 0:
                            nc.vector.tensor_copy(out=dst, in_=src)
                        else:
                            nc.scalar.copy(out=dst, in_=src)
                        nc.sync.dma_start(out=sout[hh][c2:c2 + 1],
                                          in_=dst)

            G = 0
            bi = 0
            for hh in range(2):
                qts, kts = qkts[hh]
                for t in range(NT):
                    lag = 2 if (hh, t) != (1, NT - 1) else 1
                    while len(pend) > lag:
                        colsum(pend.pop(0))
                    lhs = qts[:, 128 * t:128 * (t + 1)]
                    for c in range(NCH):
                        slot = G % RING
                        nc.tensor.matmul(ring[:, CH * slot:CH * (slot + 1)],
                                         lhs, kts[:, CH * c:CH * (c + 1)],
                                         start=True, stop=True)
                        G += 1
                        if bi < len(bnds) and G == bnds[bi]:
                            issue_window(G)
                            post_tiles()
                            bi += 1
            while pend:
                colsum(pend.pop(0))

    nc.compile()
    return nc


def _get_compiled():
    global _COMPILED
    if _COMPILED is None:
        _COMPILED = _build()
    return _COMPILED


def make_in_maps(x, Wq, bq, Wk, bk, Wv, pe):
    """Host-side sharding: build the per-core input dicts."""
    import ml_dtypes

    x = np.asarray(x, np.float32)
    Wq = np.asarray(Wq, np.float32)
    bq = np.asarray(bq, np.float32).reshape(H, D)
    Wk = np.asarray(Wk, np.float32)
    bk = np.asarray(bk, np.float32).reshape(H, D)
    Wv = np.asarray(Wv, np.float32)
    pe = np.asarray(pe, np.float32)

    xq = x + pe[None, :, :]                       # (B, L, C)
    v = np.einsum("blc,ch->blh", x, Wv)           # (B, L, H)
    q_all = (xq @ Wq).reshape(B, L, H, D) + bq[None, None]   # (B, L, H, D)
    k_all = (xq @ Wk).reshape(B, L, H, D) + bk[None, None]

    p_idx = np.arange(128)
    j_idx = np.arange(512)
    msk = np.zeros((128, 4, 512), np.float32)
    for r in range(4):
        msk[:, r, :] = (j_idx[None, :] <= 128 * r + p_idx[:, None])
    msk = msk.astype(ml_dtypes.bfloat16)

    in_maps = []
    for core in range(NCORES):
        b = core // 4
        h0 = 2 * (core % 4)
        qk4 = np.empty((4, 32, L), np.float32)
        for hh in range(2):
            qk4[2 * hh] = q_all[b, :, h0 + hh, :].T
            qk4[2 * hh + 1] = k_all[b, :, h0 + hh, :].T
        qk4 = qk4.astype(ml_dtypes.bfloat16)
        vnat = np.empty((128, 2 * NT), np.float32)
        for hh in range(2):
            # vnat[p, NT*hh + t] = v[b, 128*t + p, h0+hh]
            vnat[:, NT * hh:NT * (hh + 1)] = v[b, :, h0 + hh].reshape(NT, 128).T
        in_maps.append(dict(qk4=qk4, vnat=vnat, msk=msk))
    return in_maps


def postprocess(results):
    """Host-side gather: W=3 same-pool, assemble (B, L, H)."""
    S = np.zeros((H, B, L), np.float32)
    for core in range(NCORES):
        b = core // 4
        h0 = 2 * (core % 4)
        sraw = np.asarray(results[core]["sout"], np.float32)  # (2, 4, 512)
        for hh in range(2):
            S[h0 + hh, b, :] = sraw[hh].reshape(L)
    Sp = np.pad(S, ((0, 0), (0, 0), (1, 1)))
    sums = Sp[:, :, :-2] + Sp[:, :, 1:-1] + Sp[:, :, 2:]
    counts = np.full(L, float(W), np.float32)
    counts[0] = counts[-1] = W - 1
    pooled = sums / counts[None, None, :]
    return np.ascontiguousarray(pooled.transpose(1, 2, 0)).astype(np.float32)


def kernel(x, Wq, bq, Wk, bk, Wv, pe):
    global LAST_EXEC_NS
    from concourse.bass_utils import run_bass_kernel_spmd

    nc = _get_compiled()
    in_maps = make_in_maps(x, Wq, bq, Wk, bk, Wv, pe)
    res = run_bass_kernel_spmd(nc, in_maps, list(range(NCORES)), trace=TRACE)
    LAST_EXEC_NS = res.exec_time_ns
    return postprocess(res.results)


# revision 11
# speedup vs baseline: 1.0455x; 1.0455x over previous
"""Trainium2 Bass kernel for nn_MultiHeadDistanceLayer.

Math: out[b,k,h] = pool3(S[h,b,:])[k] where
  S[h,b,k'] = sum_{q>=k'} v[h,b,q] * softmax(QK^T/sqrt(D))[q,k']
(the final sum over the query axis commutes with the W=3 key-axis average
pool, so the device only produces the length-L vector S per (head, batch);
pooling/normalization is a trivial host epilogue).

Sharding: 16 (head, batch) pairs; core c handles batch c//4 and heads
(2*(c%4), 2*(c%4)+1). The tiny O(L*C*D) Q/K/v projections run on the host
(0.8% of FLOPs); the device does the O(L^2) work: scores, softmax, and
causal weighted column sums.

Device pipeline (v2, ScalarE-bound design): PSUM is organized as a 7-bank
ring [128, 3584] that the score matmuls (bf16, contraction zero-padded to
K=128 for the HAM clock gate) fill in 512-col chunks, while ScalarE drains
it with large alternating [128,1536]/[128,2048] exp instructions into a
single contiguous bf16 E buffer (this amortizes the ~500-cycle per-
instruction ACT overhead that dominates a per-tile 2x[128,1024] scheme).
The softmax denominator Z is computed entirely off ScalarE: a 1-input DVE
tensor_scalar copy with accum_out sums each row at the 2x bf16 rate.
Causal column sums are M=1 matmuls (lhsT = v/Z column) accumulating in the
8th PSUM bank, lagged two tiles so the PE never stalls on the DVE w-chain.
"""

import sys

for _p in ("/opt/trn_rl_repo",):
    if _p not in sys.path:
        sys.path.insert(0, _p)

import numpy as np

B, L, C = 2, 2048, 256
H, D, W = 8, 32, 3
NCORES = 8
NT = L // 128          # 16 q-tiles per head
NCH = L // 512         # 4 key chunks per row
SCALE = float(D) ** -0.5
CH = 512
RING = 7               # PSUM ring slots (banks); bank 8 = colsum acc

TRACE = False
LAST_EXEC_NS = None
_COMPILED = None


def _window_boundaries():
    """Exp-window end boundaries over the 128-chunk global stream:
    alternating 3/4-chunk windows aligned to the 7-slot ring, final
    partial window at 128."""
    bnds = []
    b = 0
    while b < 126:
        b += 3
        bnds.append(b)
        b += 4
        bnds.append(b)
    bnds.append(128)
    return bnds


def _build():
    import concourse.bacc as bacc
    import concourse.tile as tile
    from concourse import mybir

    f32 = mybir.dt.float32
    bf16 = mybir.dt.bfloat16
    AF = mybir.ActivationFunctionType
    ALU = mybir.AluOpType

    nc = bacc.Bacc("TRN2", target_bir_lowering=False, debug=False,
                   num_devices=NCORES)

    # host-projected Q/K, transposed + bf16: rows [QT_h0, KT_h0, QT_h1, KT_h1]
    qk4 = nc.dram_tensor("qk4", [4, 32, L], bf16, kind="ExternalInput")
    vnat = nc.dram_tensor("vnat", [128, 2 * NT], f32, kind="ExternalInput")
    # 4 causal masks for the diagonal-containing 512-chunk; variant r=t%4
    # keeps column j (of the chunk) iff j <= 128*r + p.
    msk = nc.dram_tensor("msk", [128, 4, 512], bf16, kind="ExternalInput")
    # per head: acc rows 32c hold S for key chunk c
    sout = nc.dram_tensor("sout", [2, 4, 512], f32, kind="ExternalOutput")

    bnds = _window_boundaries()

    with tile.TileContext(nc) as tc:
        with (
            tc.tile_pool(name="big", bufs=1) as big,
            tc.tile_pool(name="qkp", bufs=2) as qkp,
            tc.tile_pool(name="zpool", bufs=2) as zpool,
            tc.tile_pool(name="empool", bufs=4) as empool,
            tc.tile_pool(name="small", bufs=16) as small,
            tc.tile_pool(name="ssbp", bufs=2) as ssbp,
            tc.tile_pool(name="pring", bufs=1, space="PSUM") as pring,
            tc.tile_pool(name="psacc", bufs=1, space="PSUM") as psacc,
        ):
            # --- per-head K-padded Q/K scratch, zeroed first (rows 32+
            # must be zero; K=32 matmuls do not register as PE activity
            # for the HAM clock gate, K=128 do) ---
            qkts = []
            for hh in range(2):
                qts = qkp.tile([128, L], bf16, tag=f"qts{hh}", name=f"qts{hh}")
                kts = qkp.tile([128, L], bf16, tag=f"kts{hh}", name=f"kts{hh}")
                qkts.append((qts, kts))

            # --- exp table preload (hidden under input DMA) ---
            warm = big.tile([128, 1], f32, tag="warm")
            nc.vector.memset(warm, 0.0)
            nc.scalar.activation(out=warm, in_=warm, func=AF.Exp)
            # --- PE warmup: ~4us of dense K=128 matmuls during the DMA
            # wait trips the HAM activity window so the first real tiles
            # run at 2.4GHz instead of 1.2GHz
            wrmt = big.tile([128, 512], bf16, tag="wrmt")
            nc.gpsimd.memset(wrmt.bitcast(mybir.dt.uint32), 0)
            wrmp = psacc.tile([128, 512], f32, tag="acc", name="wrmp")
            for i in range(9):
                nc.tensor.matmul(wrmp, wrmt[:, 0:128], wrmt,
                                 start=True, stop=True)

            # --- zero the K-pad rows, then DMA Q/K into rows 0-31 ---
            nc.vector.memset(qkts[0][1].bitcast(mybir.dt.uint32), 0)
            nc.gpsimd.memset(qkts[0][0].bitcast(mybir.dt.uint32), 0)
            nc.sync.dma_start(out=qkts[0][1][0:32, :], in_=qk4[1])
            nc.scalar.dma_start(out=qkts[0][0][0:32, :], in_=qk4[0])
            nc.vector.memset(qkts[1][1].bitcast(mybir.dt.uint32), 0)
            nc.gpsimd.memset(qkts[1][0].bitcast(mybir.dt.uint32), 0)
            nc.sync.dma_start(out=qkts[1][1][0:32, :], in_=qk4[3])
            nc.scalar.dma_start(out=qkts[1][0][0:32, :], in_=qk4[2])
            vnat_sb = big.tile([128, 2 * NT], f32, tag="vnat")
            nc.gpsimd.dma_start(out=vnat_sb, in_=vnat[:, :])
            msk_sb = big.tile([128, 4, 512], bf16, tag="msk")
            nc.gpsimd.dma_start(out=msk_sb, in_=msk[:, :, :])
            ones32 = big.tile([128, 32], bf16, tag="ones32")
            nc.vector.memset(ones32, 1.0)

            # --- big shared buffers ---
            ering = big.tile([128, 128 * CH], bf16, tag="ering")  # exp(scores)
            ring = pring.tile([128, RING * CH], f32, tag="ring")
            acc = psacc.tile([128, 512], f32, tag="acc", name="acc")

            st = {"ws": 0, "tl_post": 0}
            pend = []

            def issue_window(we):
                s0 = st["ws"] % RING
                n = we - st["ws"]
                nc.scalar.activation(
                    out=ering[:, CH * st["ws"]:CH * we],
                    in_=ring[:, CH * s0:CH * (s0 + n)],
                    func=AF.Exp, scale=SCALE)
                st["ws"] = we

            def post_tiles():
                # per-tile epilogue for tiles fully covered by issued windows
                while st["tl_post"] * 4 + 4 <= st["ws"]:
                    tl = st["tl_post"]
                    st["tl_post"] += 1
                    hh, t = tl // NT, tl % NT
                    et = ering[:, CH * 4 * tl:CH * 4 * (tl + 1)]
                    # Z: pairwise TT pre-add at the 2x bf16 rate halves the
                    # elements the 1x-rate cache_reduce has to stream
                    zscr = zpool.tile([128, L // 2], bf16, tag="zscr")
                    zscr2 = zpool.tile([128, L // 2], bf16, tag="zscr2")
                    z = small.tile([128, 1], f32, tag="z")
                    nc.vector.tensor_tensor(zscr, et[:, 0:L // 2],
                                            et[:, L // 2:L], op=ALU.add)
                    nc.vector.tensor_scalar(out=zscr2, in0=zscr, scalar1=1.0,
                                            scalar2=None, op0=ALU.mult,
                                            op1=ALU.add, accum_out=z)
                    zr = small.tile([128, 1], f32, tag="zr")
                    nc.vector.reciprocal(zr, z)
                    # w replicated to 32 cols: M=32 colsum keeps the PE's
                    # HAM activity metric high (M=1 throttles the clock)
                    w = small.tile([128, 32], bf16, tag="w")
                    nc.vector.tensor_scalar(out=w, in0=ones32,
                                            scalar1=vnat_sb[:, tl:tl + 1],
                                            scalar2=zr, op0=ALU.mult,
                                            op1=ALU.mult)
                    cb = t // 4
                    em = empool.tile([128, 512], bf16, tag="em")
                    nc.gpsimd.tensor_mul(
                        em, ering[:, CH * (4 * tl + cb):CH * (4 * tl + cb + 1)],
                        msk_sb[:, t % 4, :])
                    pend.append((hh, t, w, em))

            def colsum(work):
                hh, t, w, em = work
                cb = t // 4
                for c2 in range(cb + 1):
                    g = 64 * hh + 4 * t + c2
                    rhs = em if c2 == cb else ering[:, CH * g:CH * (g + 1)]
                    nc.tensor.matmul(acc[32 * c2:32 * (c2 + 1), :], w, rhs,
                                     start=(t == 4 * c2), stop=(t == NT - 1),
                                     tile_position=(0, 32 * c2),
                                     skip_group_check=True)
                if t == NT - 1:
                    # copy the 4 written acc rows out and ship; split across
                    # DVE and ACT so the end-of-kernel tail stays short
                    ssb = ssbp.tile([128, 512], f32, tag="ssb")
                    for c2 in range(NCH):
                        src = acc[32 * c2:32 * c2 + 1, :]
                        dst = ssb[32 * c2:32 * c2 + 1, :]
                        if c2 % 2 == 0:
                            nc.vector.tensor_copy(out=dst, in_=src)
                        else:
                            nc.scalar.copy(out=dst, in_=src)
                        nc.sync.dma_start(out=sout[hh][c2:c2 + 1],
                                          in_=dst)

            G = 0
            bi = 0
            for hh in range(2):
                qts, kts = qkts[hh]
                for t in range(NT):
                    lag = 2 if (hh, t) != (1, NT - 1) else 1
                    while len(pend) > lag:
                        colsum(pend.pop(0))
                    lhs = qts[:, 128 * t:128 * (t + 1)]
                    for c in range(NCH):
                        slot = G % RING
                        nc.tensor.matmul(ring[:, CH * slot:CH * (slot + 1)],
                                         lhs, kts[:, CH * c:CH * (c + 1)],
                                         start=True, stop=True)
                        G += 1
                        if bi < len(bnds) and G == bnds[bi]:
                            issue_window(G)
                            post_tiles()
                            bi += 1
            while pend:
                colsum(pend.pop(0))

    nc.compile()
    return nc


def _get_compiled():
    global _COMPILED
    if _COMPILED is None:
        _COMPILED = _build()
    return _COMPILED


def make_in_maps(x, Wq, bq, Wk, bk, Wv, pe):
    """Host-side sharding: build the per-core input dicts."""
    import ml_dtypes

    x = np.asarray(x, np.float32)
    Wq = np.asarray(Wq, np.float32)
    bq = np.asarray(bq, np.float32).reshape(H, D)
    Wk = np.asarray(Wk, np.float32)
    bk = np.asarray(bk, np.float32).reshape(H, D)
    Wv = np.asarray(Wv, np.float32)
    pe = np.asarray(pe, np.float32)

    xq = x + pe[None, :, :]                       # (B, L, C)
    v = np.einsum("blc,ch->blh", x, Wv)           # (B, L, H)
    q_all = (xq @ Wq).reshape(B, L, H, D) + bq[None, None]   # (B, L, H, D)
    k_all = (xq @ Wk).reshape(B, L, H, D) + bk[None, None]

    p_idx = np.arange(128)
    j_idx = np.arange(512)
    msk = np.zeros((128, 4, 512), np.float32)
    for r in range(4):
        msk[:, r, :] = (j_idx[None, :] <= 128 * r + p_idx[:, None])
    msk = msk.astype(ml_dtypes.bfloat16)

    in_maps = []
    for core in range(NCORES):
        b = core // 4
        h0 = 2 * (core % 4)
        qk4 = np.empty((4, 32, L), np.float32)
        for hh in range(2):
            qk4[2 * hh] = q_all[b, :, h0 + hh, :].T
            qk4[2 * hh + 1] = k_all[b, :, h0 + hh, :].T
        qk4 = qk4.astype(ml_dtypes.bfloat16)
        vnat = np.empty((128, 2 * NT), np.float32)
        for hh in range(2):
            # vnat[p, NT*hh + t] = v[b, 128*t + p, h0+hh]
            vnat[:, NT * hh:NT * (hh + 1)] = v[b, :, h0 + hh].reshape(NT, 128).T
        in_maps.append(dict(qk4=qk4, vnat=vnat, msk=msk))
    return in_maps


def postprocess(results):
    """Host-side gather: W=3 same-pool, assemble (B, L, H)."""
    S = np.zeros((H, B, L), np.float32)
    for core in range(NCORES):
        b = core // 4
        h0 = 2 * (core % 4)
        sraw = np.asarray(results[core]["sout"], np.float32)  # (2, 4, 512)
        for hh in range(2):
            S[h0 + hh, b, :] = sraw[hh].reshape(L)
    Sp = np.pad(S, ((0, 0), (0, 0), (1, 1)))
    sums = Sp[:, :, :-2] + Sp[:, :, 1:-1] + Sp[:, :, 2:]
    counts = np.full(L, float(W), np.float32)
    counts[0] = counts[-1] = W - 1
    pooled = sums / counts[None, None, :]
    return np.ascontiguousarray(pooled.transpose(1, 2, 0)).astype(np.float32)


def kernel(x, Wq, bq, Wk, bk, Wv, pe):
    global LAST_EXEC_NS
    from concourse.bass_utils import run_bass_kernel_spmd

    nc = _get_compiled()
    in_maps = make_in_maps(x, Wq, bq, Wk, bk, Wv, pe)
    res = run_bass_kernel_spmd(nc, in_maps, list(range(NCORES)), trace=TRACE)
    LAST_EXEC_NS = res.exec_time_ns
    return postprocess(res.results)
